# revision 47
# baseline (speedup 1.0000x reference)
"""Multi-head attention (B=4, T=2048, D=1024, H=16) on 8 TRN2 NeuronCores.

Sharding: core c -> (batch b = c//2, head-group g = c%2 of 8 heads).
Each core computes the qkv projection for its batch restricted to its 8
heads, full attention for those heads, and a partial output projection
(ctx_local @ Wout[rows of its heads]).  Host sums the two partials per batch.

All device inputs are pre-cast to bf16 on the host.  Per-core kernel,
organized so the PE stream is dense and ACT (softmax exp) saturated:

  x kept fully resident in SBUF (loaded once, 4 early DMAs on separate
  engine queues, ordered by first use: x-span0, wq, wk, rest of x, wv,
  wout last);
  qk-projection for head-pair 0, then v-projection (all heads),
  then for each head-pair hc: attention for both heads over all query
  quarters (S pairs = two row-tiled concurrent matmuls, one per head;
  exp on ACT [128,1024] PSUM->SBUF; ctx^T+sumexp via [v|1].T @ P;
  normalization via DVE reciprocal + gpsimd partition broadcast),
  interleaved with the qk-projection of the next pair; during the last
  pair, the output projection runs per query quarter.

PSUM: spsum 2x2 banks, ctx ring 3x1 banks (so the norm chain of quarter
q never blocks quarter q+1's AV), proj 1 bank.  P2 (exp output) is a
4-chunk ring (AV trails exp by one chunk).
"""

import numpy as np
import ml_dtypes
from contextlib import ExitStack

import concourse.bass as bass
import concourse.bacc as bacc
import concourse.tile as tile
from concourse import mybir
from concourse.bass_utils import run_bass_kernel_spmd
from concourse.tile_rust import add_dep_helper

FP32 = mybir.dt.float32
BF16 = mybir.dt.bfloat16
EXP = mybir.ActivationFunctionType.Exp

D = 1024
T = 2048
HPC = 8          # heads per core
FC = 8           # feature chunks of 128 (projection contraction)
TS = 4           # token spans of 512
KC = 16          # k chunks of 128
QQ = 4           # query quarters of 512
PR = 12          # P2 ring depth (chunks): AV(kc) must be emitted before
                 # exp(kc+12) of the same block (two P2 tiles alternate
                 # across blocks).

# Within-block AV emission schedule, keyed by kc-pair GROUP index g (kc =
# 2g, 2g+1).  S matmul pairs for the two kc of a group are emitted
# back-to-back: entering/leaving the PE's 64-row quadrant mode costs ~118 ns
# per switch, so batching two S-pairs halves the transition count.  AV for
# chunk kc is deferred ~4-5 chunks so the PE never waits on exp, then catches
# up at the block end so AV(15) (and with it the softmax-normalization chain)
# starts right after exp(15).
AV_DUE = {2: (0, 1), 3: (2, 3), 4: (4, 5), 5: (6, 7), 6: (8, 9),
          7: (10, 11, 12)}
AV_POST = (13, 14)


def _norm(nc, rpool, ctx_sb, ctxp, hh, hc, qsl):
    """ctx_sb[hb:hb+64, hc, qsl] = ctxp[0:64] / ctxp[64] (sumexp row)."""
    hb = (hh % 2) * 64
    # reciprocal_approx_fast malfunctions on a PSUM input (measured garbage);
    # stage the sumexp row through SBUF first.
    rtmp = rpool.tile([1, 512], FP32, tag="rtmp")
    nc.vector.tensor_copy(out=rtmp[:], in_=ctxp[64:65, :])
    rt = rpool.tile([1, 512], FP32, tag="rt")
    nc.vector.reciprocal_approx_fast(out=rt[:], in_=rtmp[:])
    rb = rpool.tile([64, 512], FP32, tag="rb")
    nc.gpsimd.partition_broadcast(rb[:], rt[0:1, :], channels=64)
    nc.vector.tensor_mul(ctx_sb[hb:hb + 64, hc, qsl], ctxp[0:64, :], rb[:])


def _qk_proj(nc, ps, x_sb, wq_sb, wk_sb, qT, kT, hc, ts_list=range(TS)):
    """qT/kT[:, hc, :] for head pair hc: out [dims 128, tok 512] per span."""
    for ts in ts_list:
        tsl = slice(ts * 512, (ts + 1) * 512)
        for w_sb, dst in ((wq_sb, qT), (wk_sb, kT)):
            p = ps.tile([128, 512], FP32, tag="proj")
            for fc in range(FC):
                nc.tensor.matmul(
                    p[:],
                    lhsT=w_sb[:, fc, hc * 128:(hc + 1) * 128],
                    rhs=x_sb[:, fc, tsl],
                    start=(fc == 0), stop=(fc == FC - 1))
            nc.vector.tensor_copy(out=dst[:, hc, tsl], in_=p[:])


def _attention(nc, ppool, spsum, cpsum, rpool, qT, kT, v_sb, ctx_sb, hc, qq,
               extra=None, av_due=AV_DUE, post_av=AV_POST):
    """Both heads of pair hc for query quarter qq.

    ``extra`` (called once per k-chunk) interleaves other PE work (the fused
    v-projection, the next pair's qk-projection, the output projection) into
    the ACT-bound attention stream."""
    qsl = slice(qq * 512, (qq + 1) * 512)
    P2 = ppool.tile([128, PR, 2, 512], BF16, tag="P2")
    ctxA = cpsum.tile([65, 512], FP32, tag="ctx")
    ctxB = cpsum.tile([65, 512], FP32, tag="ctx")
    def emit_av(kc):
        for i, ctxp in ((0, ctxA), (1, ctxB)):
            nc.tensor.matmul(
                ctxp[:],
                lhsT=v_sb[:, kc, 2 * hc + i, :],
                rhs=P2[:, kc % PR, i, :],
                start=(kc == 0), stop=(kc == KC - 1))

    for g in range(KC // 2):
        sps_pair = []
        for kc in (2 * g, 2 * g + 1):
            sps = spsum.tile([128, 2, 512], FP32, tag="S")
            sps_pair.append(sps)
            for i in range(2):      # head A on rows 0-63, head B on 64-127
                b0 = i * 64
                nc.tensor.matmul(
                    sps[:, i, :],
                    lhsT=kT[b0:b0 + 64, hc, kc * 128:(kc + 1) * 128],
                    rhs=qT[b0:b0 + 64, hc, qsl],
                    start=True, stop=True)
        for j, kc in enumerate((2 * g, 2 * g + 1)):
            nc.scalar.activation(
                out=P2[:, kc % PR, :, :], in_=sps_pair[j][:, :, :],
                func=EXP, scale=0.125)
        if extra is not None:
            extra(2 * g)
            extra(2 * g + 1)
        for av_kc in av_due.get(g, ()):
            emit_av(av_kc)
    for av_kc in post_av:
        emit_av(av_kc)
    emit_av(KC - 1)
    _norm(nc, rpool, ctx_sb, ctxA, 2 * hc, hc, qsl)
    _norm(nc, rpool, ctx_sb, ctxB, 2 * hc + 1, hc, qsl)


def _body(ctx, nc, tc, xt_d, wq_d, wk_d, wv_d, wo_d, out_d):
    xt_r = xt_d.rearrange("(f p) t -> p f t", p=128)
    persist = ctx.enter_context(tc.tile_pool(name="persist", bufs=1))
    qT = persist.tile([128, 4, T], BF16, tag="qT")
    kT = persist.tile([128, 4, T], BF16, tag="kT")
    v_sb = persist.tile([128, KC, HPC, 65], BF16, tag="v")
    ctx_sb = persist.tile([128, 4, T], BF16, tag="ctx")
    # cc0-2 partial sums of the last quarter's output projection (see
    # make_p3_steps): accumulated during hc3's slack, closed in the tail.
    p3part = persist.tile([128, 4, D], BF16, tag="p3part")
    wo_sb = persist.tile([128, 4, D], BF16, tag="wo")
    x_sb = persist.tile([128, FC, T], BF16, tag="x")
    wq_sb = persist.tile([128, FC, 512], BF16, tag="wq")
    wk_sb = persist.tile([128, FC, 512], BF16, tag="wk")
    wv_sb = persist.tile([128, FC, 512], BF16, tag="wv")

    # DMA schedule: three DMA queues (sync/scalar/gpsimd) at ~46 GB/s each
    # under all-core contention, FIFO within a queue.  Every x span is
    # striped across all three queues in fc-thirds so the S stream is never
    # x-gated; pair-0 wq/wk columns lead (granule-split so the first
    # projection matmul starts ~9.5us), wv arrives between spans 1 and 2
    # (v-projection runs inside block (0,0)), pair-1 columns after span 3
    # (its projection is interleaved into late qq0 / qq1-3), pairs 2-3 and
    # wout trail (needed only by hc1+ extras and hc3).
    wq_r = wq_d.rearrange("(f p) c -> p f c", p=128)
    wk_r = wk_d.rearrange("(f p) c -> p f c", p=128)
    wv_r = wv_d.rearrange("(f p) c -> p f c", p=128)
    wo_r = wo_d.rearrange("(c p) d -> p c d", p=128)
    # A DGE issue on a backed-up queue blocks its host sequencer, so the
    # scalar (ACT) queue carries exactly three small loads that drain before
    # the exp stream begins.  sync takes the [0:4] fc-halves of every span;
    # gpsimd (whose sequencer only starts norm broadcasts at block-0's end)
    # takes wk0, the [6:8] slivers, wv, and the trailing weights.
    nc.sync.dma_start(out=wq_sb[:, 0:4, 0:128], in_=wq_r[:, 0:4, 0:128])
    nc.scalar.dma_start(out=wq_sb[:, 4:8, 0:128], in_=wq_r[:, 4:8, 0:128])
    nc.gpsimd.dma_start(out=wk_sb[:, 0:4, 0:128], in_=wk_r[:, 0:4, 0:128])
    nc.gpsimd.dma_start(out=wk_sb[:, 4:8, 0:128], in_=wk_r[:, 4:8, 0:128])
    for ts in (0, 1):                # spans 0 and 1
        nc.sync.dma_start(out=x_sb[:, 0:4, ts * 512:(ts + 1) * 512],
                          in_=xt_r[:, 0:4, ts * 512:(ts + 1) * 512])
        nc.scalar.dma_start(out=x_sb[:, 4:6, ts * 512:(ts + 1) * 512],
                            in_=xt_r[:, 4:6, ts * 512:(ts + 1) * 512])
        nc.gpsimd.dma_start(out=x_sb[:, 6:8, ts * 512:(ts + 1) * 512],
                            in_=xt_r[:, 6:8, ts * 512:(ts + 1) * 512])
    nc.gpsimd.dma_start(out=wv_sb[:, 0:4, :], in_=wv_r[:, 0:4, :])
    nc.gpsimd.dma_start(out=wv_sb[:, 4:8, :], in_=wv_r[:, 4:8, :])
    for ts in (2, 3):                # spans 2 and 3 avoid the scalar queue
        nc.sync.dma_start(out=x_sb[:, 0:4, ts * 512:(ts + 1) * 512],
                          in_=xt_r[:, 0:4, ts * 512:(ts + 1) * 512])
        nc.gpsimd.dma_start(out=x_sb[:, 4:8, ts * 512:(ts + 1) * 512],
                            in_=xt_r[:, 4:8, ts * 512:(ts + 1) * 512])
    nc.sync.dma_start(out=wq_sb[:, :, 128:256], in_=wq_r[:, :, 128:256])
    nc.gpsimd.dma_start(out=wk_sb[:, :, 128:256], in_=wk_r[:, :, 128:256])
    nc.sync.dma_start(out=wq_sb[:, :, 256:512], in_=wq_r[:, :, 256:512])
    nc.gpsimd.dma_start(out=wk_sb[:, :, 256:512], in_=wk_r[:, :, 256:512])
    nc.sync.dma_start(out=wo_sb[:, 0:2, :], in_=wo_r[:, 0:2, :])
    nc.gpsimd.dma_start(out=wo_sb[:, 2:4, :], in_=wo_r[:, 2:4, :])

    nc.vector.memset(v_sb[:, :, :, 64:65], 1.0)

    # Pre-attention projections: ONLY pair 0 for spans 0-1 — the minimum for
    # the S stream of block (0,0) to start (its queries are span 0; keys for
    # kc 4-7 are span 1).  Everything else (pair-0 spans 2-3, pair 1, the
    # v-projection) is interleaved into the attention stream in DMA-arrival
    # order.
    with tc.tile_pool(name="proj0", bufs=2, space="PSUM") as ps0:
        for ts in (0, 1):
            _qk_proj(nc, ps0, x_sb, wq_sb, wk_sb, qT, kT, 0, ts_list=[ts])

    osb = ctx.enter_context(tc.tile_pool(name="osb", bufs=2))
    with tc.tile_pool(name="P", bufs=2) as ppool, \
         tc.tile_pool(name="proj", bufs=1, space="PSUM") as ps, \
         tc.tile_pool(name="spsum", bufs=2, space="PSUM") as spsum, \
         tc.tile_pool(name="cpsum", bufs=3, space="PSUM") as cpsum, \
         tc.tile_pool(name="rpool", bufs=2) as rpool:

        def vproj(kc):
            psv = ps.tile([128, 512], FP32, tag="proj")
            for fc in range(FC):
                nc.tensor.matmul(
                    psv[:],
                    lhsT=x_sb[:, fc, kc * 128:(kc + 1) * 128],
                    rhs=wv_sb[:, fc, :],
                    start=(fc == 0), stop=(fc == FC - 1))
            nc.vector.tensor_copy(
                out=v_sb[:, kc, :, 0:64],
                in_=psv[:].rearrange("p (h d) -> p h d", h=HPC))

        def make_qk_steps(next_hc, unit_order=None):
            """64 generator steps: one fc-accumulation matmul per step of the
            next pair's qk projection (4 spans x {q,k} x 8 fc)."""
            st = {"p": None}

            def step(s):
                unit, fc = divmod(s, FC)
                if unit_order is not None:
                    unit = unit_order[unit]
                ts, qk = divmod(unit, 2)
                tsl = slice(ts * 512, (ts + 1) * 512)
                w_sb, dst = ((wq_sb, qT), (wk_sb, kT))[qk]
                if fc == 0:
                    st["p"] = ps.tile([128, 512], FP32, tag="proj", name="qkp")
                nc.tensor.matmul(
                    st["p"][:],
                    lhsT=w_sb[:, fc, next_hc * 128:(next_hc + 1) * 128],
                    rhs=x_sb[:, fc, tsl],
                    start=(fc == 0), stop=(fc == FC - 1))
                if fc == FC - 1:
                    nc.vector.tensor_copy(out=dst[:, next_hc, tsl], in_=st["p"][:])
            return step

        def make_p3_steps():
            """32 steps accumulating cc0-2 of the last quarter's output
            projection into p3part (SBUF, fp32).  These only read pairs 0-2
            (done by the end of hc2), so they fill the PE slack in the hc3
            blocks; the tail then only runs the pair-3 closers."""
            st = {"po": None}

            def step(s):
                u, ph = divmod(s, 4)
                tcg, j2 = 12 + u // 2, u % 2
                if ph == 0:
                    st["po"] = ps.tile([128, 512], FP32, tag="proj", name="p3po")
                if ph < 3:
                    nc.tensor.matmul(
                        st["po"][:],
                        lhsT=ctx_sb[:, ph, tcg * 128:(tcg + 1) * 128],
                        rhs=wo_sb[:, ph, j2 * 512:(j2 + 1) * 512],
                        start=(ph == 0), stop=(ph == 2))
                else:
                    nc.vector.tensor_copy(
                        out=p3part[:, u // 2, j2 * 512:(j2 + 1) * 512],
                        in_=st["po"][:])
            return step

        def make_op_steps(qq_prev, pool=None, cc23=False):
            """16 steps emitting the output projection of qq_prev's tokens
            (4 token chunks x 2 column halves x accumulate 4 cc).  With
            ``cc23`` only pairs 2-3 are accumulated and the cc0-1 partial is
            added from its x_sb-aliased stash (see make_op01_steps)."""
            st = {"po": None, "ot": None}
            pp = pool if pool is not None else ps

            def step(s):
                unit, half = divmod(s, 2)
                tcg = qq_prev * 4 + unit // 2
                j2 = unit % 2
                if half == 0:
                    if j2 == 0:
                        st["ot"] = osb.tile([128, D], BF16, tag="ot", name="ot")
                    st["po"] = pp.tile([128, 512], FP32, tag="proj", name="po")
                    ccs = (2,) if cc23 else (0, 1)
                else:
                    ccs = (3,) if cc23 else (2, 3)
                for cc in ccs:
                    nc.tensor.matmul(
                        st["po"][:],
                        lhsT=ctx_sb[:, cc, tcg * 128:(tcg + 1) * 128],
                        rhs=wo_sb[:, cc, j2 * 512:(j2 + 1) * 512],
                        start=(cc == (2 if cc23 else 0)), stop=(cc == 3))
                if half == 1:
                    if cc23:
                        stash = OP01_STASH[qq_prev](unit)
                        ots = st["ot"][:, j2 * 512:(j2 + 1) * 512]
                        pos = st["po"][:]
                        if qq_prev == 1:    # strided [128,2,256] stash
                            ots = ots.rearrange("p (a b) -> p a b", a=2)
                            pos = pos.rearrange("p (a b) -> p a b", a=2)
                        nc.vector.tensor_add(ots, pos, stash)
                    else:
                        nc.vector.tensor_copy(
                            out=st["ot"][:, j2 * 512:(j2 + 1) * 512],
                            in_=st["po"][:])
                    if j2 == 1:
                        nc.sync.dma_start(
                            out=out_d[tcg * 128:(tcg + 1) * 128, :],
                            in_=st["ot"][:])
            return step

        # cc0-1 partial stashes live in SBUF that is dead by the time they
        # are written: wv_sb after block (0,0)'s v-projection, the pair-0/1
        # columns of wq/wk after hc0's projections, and x span 2 after
        # pair 3's span-2 units (reordered to run in block (2,0)).
        OP01_STASH = {
            0: lambda u: wv_sb[:, u, :],
            1: lambda u: (wq_sb if u < 4 else wk_sb)[
                :, 2 * (u % 4):2 * (u % 4) + 2, 0:256],
            2: lambda u: x_sb[:, u, 1024:1536],
        }

        def make_op01_steps(qq):
            """16 steps accumulating cc0-1 of quarter ``qq``'s output
            projection into its stash."""
            st = {"po": None}

            def step(s):
                u, ph = divmod(s, 2)
                tcg, j2 = qq * 4 + u // 2, u % 2
                if ph == 0:
                    st["po"] = ps.tile([128, 512], FP32, tag="proj", name="o1p")
                    for cc in (0, 1):
                        nc.tensor.matmul(
                            st["po"][:],
                            lhsT=ctx_sb[:, cc, tcg * 128:(tcg + 1) * 128],
                            rhs=wo_sb[:, cc, j2 * 512:(j2 + 1) * 512],
                            start=(cc == 0), stop=(cc == 1))
                else:
                    dst = OP01_STASH[qq](u)
                    src = st["po"][:]
                    if qq == 1:             # strided [128,2,256] stash
                        src = src.rearrange("p (a b) -> p a b", a=2)
                    nc.vector.tensor_copy(out=dst, in_=src)
            return step

        def make_budget_extra(step_fn, n_steps, n_kc, skip_last=0):
            """Spread ``n_steps`` step_fn calls evenly over ``n_kc`` kc
            iterations (holding back ``skip_last`` steps for the caller)."""
            st = {"done": 0, "kc_seen": 0}

            def extra(kc):
                st["kc_seen"] += 1
                target = min(n_steps - skip_last,
                             (st["kc_seen"] * n_steps + n_kc - 1) // n_kc)
                while st["done"] < target:
                    step_fn(st["done"])
                    st["done"] += 1
            return extra, st

        # hc0-qq0 carries pair-0's spans 2-3 (TS23: key-proj of span 2/3 must
        # be emitted before the S groups that consume them, i.e. before
        # groups 4/6; query-proj of those spans is only needed by blocks
        # qq2/qq3) and the full v-projection (VP), keyed late enough that
        # the PE reaches each step only after its DMA (wv mid-stream, x
        # spans striped) has landed.  With PR=16 the AVs all defer to the
        # last two groups, after wv + vproj.
        qk0_step = make_qk_steps(0)
        qk1_step = make_qk_steps(1)
        TS23 = {6: range(40, 48), 10: range(56, 64),
                14: range(32, 40), 15: range(48, 56)}
        VP = {9: (0, 1), 11: (2, 3), 12: (4, 5, 6), 13: (7, 8, 9),
              14: (10, 11, 12), 15: (13, 14, 15)}
        AV_DUE_QQ0 = {5: (0, 1), 6: (2, 3), 7: (4, 5)}

        def qq0_extra(kc):
            for s in TS23.get(kc, ()):
                qk0_step(s)
            for j in VP.get(kc, ()):
                vproj(j)

        op01_q0 = make_op01_steps(0)
        op01_q1 = make_op01_steps(1)
        op01_q2 = make_op01_steps(2)
        for hc in range(4):
            # hc0: vproj fills qq0, pair-1 qk spread over qq1-3 (48 kc).
            # hc1/2: next pair's qk spread over all four quarters (64 kc) so
            # each block's PE rate just matches the ACT (exp) rate.
            # hc3: output projection of the previous quarter, one step per kc;
            # for the last quarter 4 steps are held back and emitted after the
            # norms, so the PE has work while the final norm chain runs.
            qk_extra = qk_st = None
            if hc == 0:
                # pair 1's whole projection spreads over qq1-3
                qk_extra, qk_st = make_budget_extra(qk1_step, 64, 48)
            elif hc < 3:
                # pair 3's span-2 units run first (block (2,0)) so x span 2
                # is dead before quarter 2's cc0-1 stash overwrites it
                qk_step = make_qk_steps(
                    hc + 1, unit_order=(4, 5, 0, 1, 2, 3, 6, 7) if hc == 2
                    else None)
                qk_extra, qk_st = make_budget_extra(qk_step, 64, 64)
            else:
                p3_step = make_p3_steps()
            for qq in range(QQ):
                post = None
                if hc == 0 and qq == 0:
                    _attention(nc, ppool, spsum, cpsum, rpool,
                               qT, kT, v_sb, ctx_sb, 0, 0, extra=qq0_extra,
                               av_due=AV_DUE_QQ0,
                               post_av=(6, 7, 8, 9, 10, 11, 12, 13, 14))
                    continue
                if hc == 0:
                    extra = qk_extra
                elif hc in (1, 2):
                    # hc1/hc2 slack absorbs the cc0-1 halves of quarters
                    # 0-2's output projection (pairs 0-1 ctx is final a
                    # whole hc earlier)
                    o1 = {(1, 1): (op01_q0, 0), (1, 2): (op01_q0, 8),
                          (1, 3): (op01_q1, 0), (2, 0): (op01_q1, 8),
                          (2, 1): (op01_q2, 0), (2, 2): (op01_q2, 8),
                          }.get((hc, qq))
                    o1_in = None
                    if o1 is not None:
                        o1_in, _ = make_budget_extra(
                            lambda i, f=o1[0], b=o1[1]: f(b + i), 8, 16)

                    def extra(kc, o1_in=o1_in):
                        qk_extra(kc)
                        if o1_in is not None:
                            o1_in(kc)
                else:
                    # p3 budget per hc3 block: 12, 12, 8, 0 of the 32 steps
                    p3_n, p3_base = ((12, 0), (12, 12), (8, 24), (0, 32))[qq]
                    p3_in = None
                    if p3_n:
                        p3_in, _ = make_budget_extra(
                            lambda i, b=p3_base, f=p3_step: f(b + i), p3_n, 16)
                    op_in = None
                    hold = 0
                    if qq >= 1:
                        op_step = make_op_steps(qq - 1, cc23=True)
                        hold = 16 if qq == 3 else 0
                        # skip the first 3 chunks: the previous quarter's norm
                        # (which this projection reads) lands ~3 chunks in
                        op_in, op_st = make_budget_extra(op_step, 16, 13,
                                                         skip_last=hold)

                    def extra(kc, op_in=op_in, p3_in=p3_in):
                        if op_in is not None and kc >= 3:
                            op_in(kc)
                        if p3_in is not None:
                            p3_in(kc)

                    if hold:
                        def post(op_step=op_step, op_st=op_st):
                            while op_st["done"] < 16:
                                op_step(op_st["done"])
                                op_st["done"] += 1
                _attention(nc, ppool, spsum, cpsum, rpool,
                           qT, kT, v_sb, ctx_sb, hc, qq, extra=extra)
                if post is not None:
                    post()

        # Tail: only the pair-3 (cc3) closers remain — cc0-2 partials were
        # accumulated into p3part during the hc3 blocks.  Each closer is a
        # single-shot matmul into a spsum/proj bank (free once exp(14)/15 of
        # the last block are done), then DVE adds the partial and the chunk
        # goes out.  The PE bridges the final norm chain with the 12
        # held-back qq2 projection steps emitted just above.
        dma_engs = (nc.sync, nc.scalar, nc.gpsimd, nc.sync)
        st = {"ot": None}

        def tail_close(u, po):
            tcg, j2 = 12 + u // 2, u % 2
            nc.tensor.matmul(
                po[:],
                lhsT=ctx_sb[:, 3, tcg * 128:(tcg + 1) * 128],
                rhs=wo_sb[:, 3, j2 * 512:(j2 + 1) * 512],
                start=True, stop=True)
            if j2 == 0:
                st["ot"] = osb.tile([128, D], BF16, tag="ot", name="ot")
            nc.vector.tensor_add(
                st["ot"][:, j2 * 512:(j2 + 1) * 512], po[:],
                p3part[:, u // 2, j2 * 512:(j2 + 1) * 512])
            if u == 6:
                # the last chunk's halves go out as separate DMAs on two
                # queues so the final transfer is half as long
                nc.scalar.dma_start(out=out_d[tcg * 128:(tcg + 1) * 128, 0:512],
                                    in_=st["ot"][:, 0:512])
            elif u == 7:
                nc.sync.dma_start(out=out_d[tcg * 128:(tcg + 1) * 128, 512:1024],
                                  in_=st["ot"][:, 512:1024])
            elif j2 == 1:
                dma_engs[u // 2].dma_start(
                    out=out_d[tcg * 128:(tcg + 1) * 128, :], in_=st["ot"][:])

        spA = spsum.tile([128, 2, 512], FP32, tag="S", name="po2")
        spB = spsum.tile([128, 2, 512], FP32, tag="S", name="po2")
        pos = [spA[:, 0, :], spA[:, 1, :], spB[:, 0, :], spB[:, 1, :],
               ps.tile([128, 512], FP32, tag="proj", name="po2")]
        for u in range(5):
            tail_close(u, pos[u])
        spC = spsum.tile([128, 2, 512], FP32, tag="S", name="po2")
        pos2 = [spC[:, 0, :], spC[:, 1, :],
                ps.tile([128, 512], FP32, tag="proj", name="po2")]
        for u in (5, 6, 7):
            tail_close(u, pos2[u - 5])


def build():
    nc = bacc.Bacc("TRN2", target_bir_lowering=False, debug=False, num_devices=8)
    xt_d = nc.dram_tensor("xt", [D, T], BF16, kind="ExternalInput").ap()
    wq_d = nc.dram_tensor("wq", [D, 512], BF16, kind="ExternalInput").ap()
    wk_d = nc.dram_tensor("wk", [D, 512], BF16, kind="ExternalInput").ap()
    wv_d = nc.dram_tensor("wv", [D, 512], BF16, kind="ExternalInput").ap()
    wo_d = nc.dram_tensor("wout", [512, D], BF16, kind="ExternalInput").ap()
    out_d = nc.dram_tensor("out", [T, D], BF16, kind="ExternalOutput").ap()
    with tile.TileContext(nc) as tc:
        with ExitStack() as ctx:
            _body(ctx, nc, tc, xt_d, wq_d, wk_d, wv_d, wo_d, out_d)
    nc.compile()
    return nc


_nc = None


def _get_nc():
    global _nc
    if _nc is None:
        _nc = build()
    return _nc


def make_in_maps(x, Wqkv, Wout):
    bf = ml_dtypes.bfloat16
    in_maps = []
    for c in range(8):
        b, g = divmod(c, 2)
        cs = slice(g * 512, (g + 1) * 512)
        in_maps.append({
            "xt": np.ascontiguousarray(x[b].T).astype(bf),
            "wq": np.ascontiguousarray(Wqkv[:, 0 * D:1 * D][:, cs]).astype(bf),
            "wk": np.ascontiguousarray(Wqkv[:, 1 * D:2 * D][:, cs]).astype(bf),
            "wv": np.ascontiguousarray(Wqkv[:, 2 * D:3 * D][:, cs]).astype(bf),
            "wout": np.ascontiguousarray(Wout[cs, :]).astype(bf),
        })
    return in_maps


def kernel(x, Wqkv, Wout, _trace=False):
    nc = _get_nc()
    x = np.asarray(x, dtype=np.float32)
    Wqkv = np.asarray(Wqkv, dtype=np.float32)
    Wout = np.asarray(Wout, dtype=np.float32)
    in_maps = make_in_maps(x, Wqkv, Wout)
    kwargs = {}
    if _trace:
        kwargs["trace"] = True
    res = run_bass_kernel_spmd(nc, in_maps, core_ids=list(range(8)), **kwargs)
    outs = [res.results[c]["out"].astype(np.float32) for c in range(8)]
    out = np.stack([outs[2 * b] + outs[2 * b + 1] for b in range(4)])
    if _trace:
        kernel.last_result = res
    return out

